# revision 1
# baseline (speedup 1.0000x reference)
"""Distributed 2-layer GAT on 8 Trainium2 NeuronCores (Bass/Tile).

Strategy (graph/data parallel, per sharding hint):
  - Nodes are sharded across 8 cores (6250 each, padded to 6272 = 49*128).
  - Within a core, nodes are greedily packed into 49 tiles of 128 so that
    per-tile in-edge counts are balanced (pad edges fill to K*128).
  - Layer tables ([h | al_src] per node) are computed locally per shard
    (x @ [W1 | W1@a_src | W1@a_dst] extended matmul) and AllGathered so
    every core holds the full node table in DRAM.
  - Edge pass per destination tile: indirect-DMA gather of source rows,
    attention weights ex = exp(leaky_relu(al_src[src]+al_dst[dst]))
    (unnormalized softmax - mathematically identical to the max-subtracted
    reference), weighted rows scatter-added into PSUM via a one-hot
    selection matmul; appended ex columns accumulate the softmax
    denominators in the same matmul. Per-node divide afterwards.
  - Dummy (pad) edges point their al_dst gather at a -1e9 row -> ex == 0.
"""

import heapq
import os
import sys
import types

import numpy as np

_BUILD_CACHE = {}


def _register_trace_hook():
    """Best-effort registration of the axon NTFF profiling hook."""
    try:
        if "antenv.axon_hooks" in sys.modules:
            return True
        from trn_agent_boot.trn_boot import _ntff_profile_via_ctypes

        hook = _ntff_profile_via_ctypes("/opt/axon/libaxon_pjrt.so")
        m = types.ModuleType("antenv.axon_hooks")
        m.get_axon_ntff_profile_hook = lambda: hook
        m.set_axon_ntff_profile_hook = lambda h: None
        sys.modules["antenv.axon_hooks"] = m
        return True
    except Exception:
        return False


def _host_prep(x, edge_index, W1, a_src1, a_dst1, b1, W2, a_src2, a_dst2, b2, C):
    x = np.asarray(x, np.float32)
    ei = np.asarray(edge_index)
    W1 = np.asarray(W1, np.float32)
    a_src1 = np.asarray(a_src1, np.float32)
    a_dst1 = np.asarray(a_dst1, np.float32)
    b1 = np.asarray(b1, np.float32)
    W2 = np.asarray(W2, np.float32)
    a_src2 = np.asarray(a_src2, np.float32)
    a_dst2 = np.asarray(a_dst2, np.float32)
    b2 = np.asarray(b2, np.float32)

    N, F = x.shape
    HEADS, HID = a_src1.shape
    D1 = HEADS * HID
    NCLS = W2.shape[1]
    assert N % C == 0
    NPC = N // C
    NT = -(-NPC // 128)
    PADN = NT * 128
    assert PADN > NPC, "need at least one pad slot per core for dummy rows"
    NPALL = C * PADN
    E = ei.shape[1]
    ET = E + N

    src = np.concatenate([ei[0], np.arange(N)]).astype(np.int64)
    dst = np.concatenate([ei[1], np.arange(N)]).astype(np.int64)

    # --- greedy degree-balanced node->tile assignment per core ---
    deg = np.bincount(dst, minlength=N).astype(np.int64)
    pos = np.empty(N, np.int64)
    for c in range(C):
        lo = c * NPC
        d = deg[lo:lo + NPC]
        order = np.argsort(-d, kind="stable")
        cnts = np.zeros(NT, np.int64)
        heap = [(0, t) for t in range(NT)]
        heapq.heapify(heap)
        ploc = np.empty(NPC, np.int64)
        for i in order:
            while True:
                load, t = heapq.heappop(heap)
                if cnts[t] < 128:
                    break
            ploc[i] = t * 128 + cnts[t]
            cnts[t] += 1
            if cnts[t] < 128:
                heapq.heappush(heap, (load + d[i], t))
        pos[lo:lo + NPC] = ploc

    ncidx = np.arange(N) // NPC
    node_at = np.full((C, PADN), -1, np.int64)
    node_at[ncidx, pos] = np.arange(N)
    grow = ncidx * PADN + pos  # global table row per node

    padrow = np.empty(C, np.int64)
    for c in range(C):
        w = np.where(node_at[c] < 0)[0]
        padrow[c] = c * PADN + w[0]

    # --- edge grouping by (dst core, dst tile) ---
    ec = dst // NPC
    et = pos[dst] // 128
    gkey = ec * NT + et
    # within each (core, tile) group, order edges by source table row so each
    # gather call's descriptors hit monotonically increasing DRAM addresses
    order_e = np.lexsort((pos[src] + (src // NPC) * PADN, gkey))
    ks = gkey[order_e]
    gstart = np.searchsorted(ks, np.arange(C * NT))
    gcnt = np.searchsorted(ks, np.arange(C * NT) + 1) - gstart
    K = int(-(-gcnt.max() // 128))
    jj = np.arange(ET) - gstart[ks]
    kk_e = jj // 128
    pp_e = jj % 128
    cc_e = ks // NT
    tt_e = ks % NT

    s_src = src[order_e]
    s_dst = dst[order_e]
    srcg = np.empty((C, NT, 128, K), np.int32)
    srcg[...] = padrow[:, None, None, None]
    dstl = np.full((C, NT, 128, K), 999.0, np.float32)
    srcg[cc_e, tt_e, pp_e, kk_e] = grow[s_src]
    dstl[cc_e, tt_e, pp_e, kk_e] = (pos[s_dst] % 128).astype(np.float32)

    # --- per-core transposed x shards (pad rows zero) ---
    xs = np.zeros((C, PADN, F), np.float32)
    xs[ncidx, pos] = x
    xsT = np.ascontiguousarray(xs.transpose(0, 2, 1))

    # --- extended weights ---
    Wa_s1 = np.einsum("fhc,hc->fh", W1.reshape(F, HEADS, HID), a_src1)
    Wa_d1 = np.einsum("fhc,hc->fh", W1.reshape(F, HEADS, HID), a_dst1)
    W1e = np.ascontiguousarray(
        np.concatenate([W1, Wa_s1, Wa_d1], axis=1), dtype=np.float32
    )
    Wa_s2 = W2 @ a_src2[0]
    Wa_d2 = W2 @ a_dst2[0]
    W2e = np.ascontiguousarray(
        np.concatenate([W2, Wa_s2[:, None], Wa_d2[:, None]], axis=1),
        dtype=np.float32,
    )

    # replicated-transposed dstl for the PE al_dst broadcast: [t, k, j, e] = dstl[t, e, k]
    dstlr = np.ascontiguousarray(
        np.broadcast_to(dstl.transpose(0, 1, 3, 2)[:, :, :, None, :],
                        (C, NT, K, 128, 128)), np.float32)
    iotac = np.arange(128, dtype=np.float32)[:, None].copy()
    b1r = np.ascontiguousarray(np.broadcast_to(b1[None, :], (128, D1)), np.float32)
    b2r = np.ascontiguousarray(np.broadcast_to(b2[None, :], (128, NCLS)), np.float32)
    iota = np.ascontiguousarray(
        np.broadcast_to(np.arange(128, dtype=np.float32)[None, :], (128, 128))
    )
    ident = np.eye(128, dtype=np.float32)

    cfg = dict(F=F, HEADS=HEADS, HID=HID, D1=D1, NCLS=NCLS, NT=NT, PADN=PADN,
               NPALL=NPALL, K=K, C=C)
    in_maps = []
    for c in range(C):
        in_maps.append({
            "xsT": xsT[c],
            "w1e": W1e,
            "w2e": W2e,
            "b1r": b1r,
            "b2r": b2r,
            "iota": iota,
            "ident": ident,
            "srcg": srcg[c],
            "dstl": dstl[c],
            "dstlr": dstlr[c],
            "iotac": iotac,
        })
    return cfg, in_maps, node_at, (N, NCLS)


def _build_program(F, HEADS, HID, D1, NCLS, NT, PADN, NPALL, K, C):
    import concourse.bacc as bacc
    import concourse.bass as bass
    import concourse.mybir as mybir
    import concourse.tile as tile

    f32 = mybir.dt.float32
    bf16 = mybir.dt.bfloat16
    i32 = mybir.dt.int32
    AF = mybir.ActivationFunctionType
    ALU = mybir.AluOpType
    AX = mybir.AxisListType

    TW1 = D1 + 2 * HEADS          # local layer-1 matmul width
    G1W = D1 + HEADS              # layer-1 gather row width [h | al_src]
    TW2 = NCLS + 2                # layer-2 table width [z2 | al_src2 | al_dst2]
    G2W = NCLS + 1                # layer-2 gather width [z2 | al_src2]
    FK = F // 128
    DK = D1 // 128

    nc = bacc.Bacc("TRN2", target_bir_lowering=False, debug=False, num_devices=C)

    xsT = nc.dram_tensor("xsT", [F, PADN], f32, kind="ExternalInput")
    w1e = nc.dram_tensor("w1e", [F, TW1], f32, kind="ExternalInput")
    w2e = nc.dram_tensor("w2e", [D1, TW2], f32, kind="ExternalInput")
    b1r = nc.dram_tensor("b1r", [128, D1], f32, kind="ExternalInput")
    b2r = nc.dram_tensor("b2r", [128, NCLS], f32, kind="ExternalInput")
    iot = nc.dram_tensor("iota", [128, 128], f32, kind="ExternalInput")
    idn = nc.dram_tensor("ident", [128, 128], f32, kind="ExternalInput")
    srcg = nc.dram_tensor("srcg", [NT, 128, K], i32, kind="ExternalInput")
    dstlr = nc.dram_tensor("dstlr", [NT, K, 128, 128], f32, kind="ExternalInput")
    iotac = nc.dram_tensor("iotac", [128, 1], f32, kind="ExternalInput")
    dstl = nc.dram_tensor("dstl", [NT, 128, K], f32, kind="ExternalInput")
    outp = nc.dram_tensor("outp", [PADN, NCLS], f32, kind="ExternalOutput")

    loc1 = nc.dram_tensor("loc1", [PADN, G1W], f32)
    tab1 = nc.dram_tensor("tab1", [NPALL, G1W], f32, addr_space="Shared")
    ald1 = nc.dram_tensor("ald1", [PADN + 1, HEADS], f32)
    loc2 = nc.dram_tensor("loc2", [PADN, TW2], f32)
    tab2 = nc.dram_tensor("tab2", [NPALL, TW2], f32, addr_space="Shared")
    ald2 = nc.dram_tensor("ald2", [PADN + 1, 1], f32)

    rg = [list(range(C))]

    with tile.TileContext(nc) as tc:
        with (
            tc.tile_pool(name="const", bufs=1) as const,
            tc.tile_pool(name="h2", bufs=1) as h2p,
            tc.tile_pool(name="shp", bufs=1) as shp,
            tc.tile_pool(name="wk", bufs=2) as wk,
            tc.tile_pool(name="idx", bufs=3) as idxp,
            tc.tile_pool(name="ps", bufs=2, space="PSUM") as psp,
        ):
            # ---- constants ----
            w1t = []
            for kk in range(FK):
                t_ = const.tile([128, TW1], f32, tag=f"w1_{kk}")
                nc.sync.dma_start(out=t_[:], in_=w1e[kk * 128:(kk + 1) * 128, :])
                w1t.append(t_)
            w2t = []
            for kk in range(DK):
                t_ = const.tile([128, TW2], f32, tag=f"w2_{kk}")
                nc.sync.dma_start(out=t_[:], in_=w2e[kk * 128:(kk + 1) * 128, :])
                w2t.append(t_)
            b1s = const.tile([128, D1], f32, tag="b1")
            nc.sync.dma_start(out=b1s[:], in_=b1r[:, :])
            b2s = const.tile([128, NCLS], f32, tag="b2")
            nc.sync.dma_start(out=b2s[:], in_=b2r[:, :])
            ios = const.tile([128, 128], f32, tag="iota")
            nc.sync.dma_start(out=ios[:], in_=iot[:, :])
            ids = const.tile([128, 128], f32, tag="ident")
            nc.sync.dma_start(out=ids[:], in_=idn[:, :])
            neg1 = const.tile([1, HEADS], f32, tag="neg1")
            nc.vector.memset(neg1[:], -1e9)
            nc.sync.dma_start(out=ald1[PADN:PADN + 1, :], in_=neg1[:])
            neg2 = const.tile([1, 1], f32, tag="neg2")
            nc.vector.memset(neg2[:], -1e9)
            nc.sync.dma_start(out=ald2[PADN:PADN + 1, :], in_=neg2[:])
            ioc = const.tile([128, 1], f32, tag="ioc")
            nc.sync.dma_start(out=ioc[:], in_=iotac[:, :])
            ssum = const.tile([128, NT], f32, tag="ssum")
            lgs = const.tile([128, NT], f32, tag="lgs")

            # ---- phase A: local h = x @ [W1 | Wa_src | Wa_dst] ----
            with nc.named_scope("l1_local_mm"):
                for t in range(NT):
                    ps_a = psp.tile([128, TW1], f32, tag="mm")
                    for kk in range(FK):
                        xt = wk.tile([128, 128], f32, tag=f"xt{kk}")
                        nc.sync.dma_start(
                            out=xt[:],
                            in_=xsT[kk * 128:(kk + 1) * 128, t * 128:(t + 1) * 128],
                        )
                        nc.tensor.matmul(ps_a[:], lhsT=xt[:], rhs=w1t[kk][:],
                                         start=(kk == 0), stop=(kk == FK - 1))
                    ha = wk.tile([128, TW1], f32, tag="ha")
                    nc.scalar.copy(ha[:], ps_a[:])
                    nc.sync.dma_start(out=loc1[t * 128:(t + 1) * 128, :],
                                      in_=ha[:, 0:G1W])
                    nc.sync.dma_start(out=ald1[t * 128:(t + 1) * 128, :],
                                      in_=ha[:, D1 + HEADS:D1 + 2 * HEADS])

            # ---- phase B: allgather layer-1 table ----
            with nc.named_scope("l1_allgather"):
                nc.gpsimd.collective_compute(
                    "AllGather", mybir.AluOpType.bypass, replica_groups=rg,
                    ins=[loc1[:]], outs=[tab1[:]],
                )
            tc.strict_bb_all_engine_barrier()

            # ---- phase C: layer-1 edge pass ----
            o1_tiles = []
            with nc.named_scope("l1_edges"):
                for t in range(NT):
                    sg = idxp.tile([128, K], i32, tag="sg")
                    nc.sync.dma_start(out=sg[:], in_=srcg[t])
                    dl = idxp.tile([128, K], f32, tag="dl")
                    nc.sync.dma_start(out=dl[:], in_=dstl[t])
                    alt = idxp.tile([128, HEADS], f32, tag="alt")
                    nc.sync.dma_start(out=alt[:], in_=ald1[t * 128:(t + 1) * 128, :])

                    g1 = wk.tile([128, K * G1W], f32, tag="g1")
                    g1v = g1[:].rearrange("p (k c) -> p k c", c=G1W)
                    for k in range(K):
                        nc.gpsimd.indirect_dma_start(
                            out=g1v[:, k, :], out_offset=None, in_=tab1[:, :],
                            in_offset=bass.IndirectOffsetOnAxis(
                                ap=sg[:, k:k + 1], axis=0),
                        )
                    # al_dst per edge via transposed-onehot matmul
                    ps_e = psp.tile([128, K * HEADS], f32, tag="ed")
                    for k in range(K):
                        dr = wk.tile([128, 128], f32, tag="dr")
                        nc.sync.dma_start(out=dr[:], in_=dstlr[t, k])
                        ohT = wk.tile([128, 128], f32, tag="ohT")
                        nc.vector.tensor_scalar(ohT[:], dr[:], ioc[:, 0:1], None,
                                                op0=ALU.is_equal)
                        nc.tensor.matmul(ps_e[:, k * HEADS:(k + 1) * HEADS],
                                         lhsT=ohT[:], rhs=alt[:],
                                         start=(k == 0), stop=(k == K - 1))

                    e1 = idxp.tile([128, K * HEADS], f32, tag="e1")
                    e1v = e1[:].rearrange("p (k h) -> p k h", h=HEADS)
                    nc.vector.tensor_add(
                        e1v, g1v[:, :, D1:D1 + HEADS],
                        ps_e[:].rearrange("p (k h) -> p k h", h=HEADS))
                    lr = idxp.tile([128, K * HEADS], f32, tag="lr")
                    nc.vector.tensor_scalar_mul(lr[:], e1[:], 0.2)
                    nc.vector.tensor_max(lr[:], lr[:], e1[:])
                    exw = idxp.tile([128, K * HEADS], f32, tag="exw")
                    nc.scalar.activation(exw[:], lr[:], AF.Exp)
                    exv = exw[:].rearrange("p (k h) -> p k h", h=HEADS)

                    # weight gathered rows in place; ex into the al_src cols
                    g1f = g1v[:, :, 0:D1].rearrange("p k (h c) -> p k h c", c=HID)
                    exb = exv.unsqueeze(3).to_broadcast([128, K, HEADS, HID])
                    nc.vector.tensor_mul(g1f, g1f, exb)
                    nc.vector.tensor_copy(g1v[:, :, D1:D1 + HEADS], exv)

                    oh = wk.tile([128, K * 128], f32, tag="oh")
                    ohv = oh[:].rearrange("p (k j) -> p k j", j=128)
                    dlb = dl[:].unsqueeze(2).to_broadcast([128, K, 128])
                    iob = ios[:].unsqueeze(1).to_broadcast([128, K, 128])
                    nc.vector.tensor_tensor(ohv, dlb, iob, op=ALU.is_equal)

                    ps_c = psp.tile([128, TW1], f32, tag="mm")
                    for k in range(K):
                        nc.tensor.matmul(
                            ps_c[:, 0:G1W],
                            lhsT=oh[:, k * 128:(k + 1) * 128],
                            rhs=g1[:, k * G1W:(k + 1) * G1W],
                            start=(k == 0), stop=(k == K - 1),
                        )

                    den = idxp.tile([128, HEADS], f32, tag="den")
                    nc.vector.tensor_scalar_add(den[:], ps_c[:, D1:D1 + HEADS], 1e-16)
                    rec = idxp.tile([128, HEADS], f32, tag="rec")
                    nc.vector.reciprocal(rec[:], den[:])

                    o1 = h2p.tile([128, D1], f32, tag=f"h2_{t}")
                    o1v = o1[:].rearrange("p (h c) -> p h c", c=HID)
                    recb = rec[:].unsqueeze(2).to_broadcast([128, HEADS, HID])
                    psf = ps_c[:, 0:D1].rearrange("p (h c) -> p h c", c=HID)
                    nc.vector.tensor_mul(o1v, psf, recb)
                    nc.vector.tensor_add(o1[:], o1[:], b1s[:])
                    # elu(x) = max(x,0) + exp(min(x,0)) - 1
                    tn = wk.tile([128, D1], f32, tag="tn")
                    nc.vector.tensor_scalar_min(tn[:], o1[:], 0.0)
                    nc.scalar.activation(tn[:], tn[:], AF.Exp)
                    nc.vector.tensor_scalar_max(o1[:], o1[:], 0.0)
                    nc.vector.tensor_add(o1[:], o1[:], tn[:])
                    nc.vector.tensor_scalar_add(o1[:], o1[:], -1.0)
                    o1_tiles.append(o1)

            # ---- phase D: layer-2 local z2 = h2 @ [W2 | Wa2_src | Wa2_dst] ----
            with nc.named_scope("l2_local_mm"):
                for t in range(NT):
                    tts = []
                    for kk in range(DK):
                        ps_t = psp.tile([128, 128], f32, tag="tr")
                        nc.tensor.transpose(
                            ps_t[:], o1_tiles[t][:, kk * 128:(kk + 1) * 128], ids[:]
                        )
                        tt = wk.tile([128, 128], f32, tag=f"tt{kk}")
                        nc.scalar.copy(tt[:], ps_t[:])
                        tts.append(tt)
                    ps_d = psp.tile([128, TW2], f32, tag="mm")
                    for kk in range(DK):
                        nc.tensor.matmul(ps_d[:], lhsT=tts[kk][:], rhs=w2t[kk][:],
                                         start=(kk == 0), stop=(kk == DK - 1))
                    hd = wk.tile([128, TW2], f32, tag="hd")
                    nc.scalar.copy(hd[:], ps_d[:])
                    nc.sync.dma_start(out=loc2[t * 128:(t + 1) * 128, :],
                                      in_=hd[:, 0:TW2])
                    nc.sync.dma_start(out=ald2[t * 128:(t + 1) * 128, :],
                                      in_=hd[:, TW2 - 1:TW2])

            # ---- phase E: allgather layer-2 table ----
            with nc.named_scope("l2_allgather"):
                nc.gpsimd.collective_compute(
                    "AllGather", mybir.AluOpType.bypass, replica_groups=rg,
                    ins=[loc2[:]], outs=[tab2[:]],
                )
            tc.strict_bb_all_engine_barrier()

            # ---- phase F: layer-2 edge pass ----
            sh_tiles = []
            with nc.named_scope("l2_edges"):
                for t in range(NT):
                    sg = idxp.tile([128, K], i32, tag="sg")
                    nc.sync.dma_start(out=sg[:], in_=srcg[t])
                    dl = idxp.tile([128, K], f32, tag="dl")
                    nc.sync.dma_start(out=dl[:], in_=dstl[t])
                    alt2 = idxp.tile([128, 1], f32, tag="alt2")
                    nc.sync.dma_start(out=alt2[:], in_=ald2[t * 128:(t + 1) * 128, :])

                    g2 = wk.tile([128, K * G2W], f32, tag="g2")
                    g2v = g2[:].rearrange("p (k c) -> p k c", c=G2W)
                    for k in range(K):
                        nc.gpsimd.indirect_dma_start(
                            out=g2v[:, k, :], out_offset=None, in_=tab2[:, :],
                            in_offset=bass.IndirectOffsetOnAxis(
                                ap=sg[:, k:k + 1], axis=0),
                        )
                    ps_e2 = psp.tile([128, K * HEADS], f32, tag="ed")
                    for k in range(K):
                        dr = wk.tile([128, 128], f32, tag="dr")
                        nc.sync.dma_start(out=dr[:], in_=dstlr[t, k])
                        ohT = wk.tile([128, 128], f32, tag="ohT")
                        nc.vector.tensor_scalar(ohT[:], dr[:], ioc[:, 0:1], None,
                                                op0=ALU.is_equal)
                        nc.tensor.matmul(ps_e2[:, k:k + 1],
                                         lhsT=ohT[:], rhs=alt2[:],
                                         start=(k == 0), stop=(k == K - 1))

                    e2 = idxp.tile([128, K], f32, tag="e2")
                    nc.vector.tensor_add(e2[:], g2v[:, :, NCLS], ps_e2[:, 0:K])
                    lr2 = idxp.tile([128, K], f32, tag="lr2")
                    nc.vector.tensor_scalar_mul(lr2[:], e2[:], 0.2)
                    nc.vector.tensor_max(lr2[:], lr2[:], e2[:])
                    ex2 = idxp.tile([128, K], f32, tag="ex2")
                    nc.scalar.activation(ex2[:], lr2[:], AF.Exp)

                    g2f = g2v[:, :, 0:NCLS]
                    ex2b = ex2[:].unsqueeze(2).to_broadcast([128, K, NCLS])
                    nc.vector.tensor_mul(g2f, g2f, ex2b)
                    nc.vector.tensor_copy(g2v[:, :, NCLS], ex2[:])

                    oh = wk.tile([128, K * 128], f32, tag="oh")
                    ohv = oh[:].rearrange("p (k j) -> p k j", j=128)
                    dlb = dl[:].unsqueeze(2).to_broadcast([128, K, 128])
                    iob = ios[:].unsqueeze(1).to_broadcast([128, K, 128])
                    nc.vector.tensor_tensor(ohv, dlb, iob, op=ALU.is_equal)

                    ps_f = psp.tile([128, TW1], f32, tag="mm")
                    for k in range(K):
                        nc.tensor.matmul(
                            ps_f[:, 0:G2W],
                            lhsT=oh[:, k * 128:(k + 1) * 128],
                            rhs=g2[:, k * G2W:(k + 1) * G2W],
                            start=(k == 0), stop=(k == K - 1),
                        )

                    den2 = idxp.tile([128, 1], f32, tag="den2")
                    nc.vector.tensor_scalar_add(den2[:], ps_f[:, NCLS:NCLS + 1], 1e-16)
                    rec2 = idxp.tile([128, 1], f32, tag="rec2")
                    nc.vector.reciprocal(rec2[:], den2[:])

                    o2 = wk.tile([128, NCLS], f32, tag="o2")
                    nc.vector.tensor_scalar(o2[:], ps_f[:, 0:NCLS], rec2[:], None,
                                            op0=ALU.mult)
                    nc.vector.tensor_add(o2[:], o2[:], b2s[:])

                    rmax = idxp.tile([128, 1], f32, tag="rmax")
                    nc.vector.reduce_max(rmax[:], o2[:], axis=AX.X)
                    sh = shp.tile([128, NCLS], f32, tag=f"sh_{t}")
                    nc.vector.tensor_scalar(sh[:], o2[:], rmax[:], None,
                                            op0=ALU.subtract)
                    exs = wk.tile([128, NCLS], f32, tag="exs")
                    nc.scalar.activation(exs[:], sh[:], AF.Exp)
                    nc.vector.reduce_sum(ssum[:, t:t + 1], exs[:], axis=AX.X)
                    sh_tiles.append(sh)

            # ---- phase G: log-softmax finalize ----
            with nc.named_scope("logsoftmax"):
                nc.scalar.activation(lgs[:], ssum[:], AF.Ln)
                for t in range(NT):
                    outf = wk.tile([128, NCLS], f32, tag="outf")
                    nc.vector.tensor_scalar(outf[:], sh_tiles[t][:], lgs[:, t:t + 1],
                                            None, op0=ALU.subtract)
                    nc.sync.dma_start(out=outp[t * 128:(t + 1) * 128, :], in_=outf[:])

    nc.compile()
    return nc


def _get_program(cfg):
    key = tuple(sorted(cfg.items()))
    if key not in _BUILD_CACHE:
        _BUILD_CACHE[key] = _build_program(**cfg)
    return _BUILD_CACHE[key]


def kernel(**inputs):
    C = 8
    cfg, in_maps, node_at, (N, NCLS) = _host_prep(
        inputs["x"], inputs["edge_index"], inputs["W1"], inputs["a_src1"],
        inputs["a_dst1"], inputs["b1"], inputs["W2"], inputs["a_src2"],
        inputs["a_dst2"], inputs["b2"], C,
    )
    nc = _get_program(cfg)

    from concourse.bass_utils import run_bass_kernel_spmd

    trace = bool(int(os.environ.get("GAT_PROFILE", "0")))
    if trace:
        trace = _register_trace_hook()
    res = run_bass_kernel_spmd(nc, in_maps, list(range(C)), trace=trace)
    if trace and res.exec_time_ns is not None:
        print(f"HW exec time: {res.exec_time_ns} ns", flush=True)

    out = np.empty((N, NCLS), np.float32)
    for c in range(C):
        r = res.results[c]["outp"]
        m = node_at[c] >= 0
        out[node_at[c][m]] = r[m]
    return out



# revision 22
# speedup vs baseline: 3.7597x; 3.7597x over previous
"""Distributed 2-layer GAT on 8 Trainium2 NeuronCores (Bass/Tile).

Strategy (graph/data parallel, per sharding hint):
  - Nodes are placed so that each edge slot's partition equals its
    destination's row within the destination tile: al_dst becomes a free
    broadcast and the segment scatter-add is a PSUM accumulation with an
    identity lhsT (no one-hot matmuls).
  - Source nodes are split into two tables A/B (each < 32768 rows so
    int16 bulk-gather indices work).  A host-side discrepancy optimizer
    balances every destination's in-edges across A/B so the ragged
    per-tile K schedules stay dense (~88% slot fill).
  - Node tables ([h | al_src | pad] per node, bf16, 768B rows) are
    computed per shard (x @ [W1 | W1@a_src | W1@a_dst] extended matmul)
    and AllGathered so every core holds both full tables in DRAM.
  - Per destination tile: two bulk dma_gather calls (InstDMAGatherAnt,
    one per table) pull all source rows; ex = exp(leaky_relu(al_src +
    al_dst)) (unnormalized softmax); rows are weighted by ex and
    accumulated in PSUM via identity matmuls, with ex riding in the
    al_src columns to produce the softmax denominators.
  - Pad slots gather a row whose al_src is poked to -1e9 -> ex == 0.
"""

import os
import sys
import types

import numpy as np
from ml_dtypes import bfloat16

_BUILD_CACHE = {}


def _register_trace_hook():
    """Best-effort registration of the axon NTFF profiling hook."""
    try:
        if "antenv.axon_hooks" in sys.modules:
            return True
        from trn_agent_boot.trn_boot import _ntff_profile_via_ctypes

        hook = _ntff_profile_via_ctypes("/opt/axon/libaxon_pjrt.so")
        m = types.ModuleType("antenv.axon_hooks")
        m.get_axon_ntff_profile_hook = lambda: hook
        m.set_axon_ntff_profile_hook = lambda h: None
        sys.modules["antenv.axon_hooks"] = m
        return True
    except Exception:
        return False


def _balance_split(src, dst, deg, N, target, lo, hi):
    """Assign each node to table A/B so every dst's in-edges split evenly."""
    rng = np.random.default_rng(0)
    inA = np.zeros(N, bool)
    inA[rng.permutation(N)[:target]] = True
    for it in range(40):
        rho = 0.5 if it < 10 else (0.3 if it < 25 else 0.15)
        dA = np.bincount(dst, weights=inA[src].astype(np.float64), minlength=N)
        diff = 2 * dA - deg
        per_edge = diff[dst]
        gAB = np.bincount(src, weights=(4.0 - 4.0 * per_edge), minlength=N)
        gBA = np.bincount(src, weights=(4.0 + 4.0 * per_edge), minlength=N)
        fA = inA & (gAB < 0) & (rng.random(N) < rho)
        fB = (~inA) & (gBA < 0) & (rng.random(N) < rho)
        na = inA.sum() - fA.sum() + fB.sum()
        if na > hi:
            drop = int(na - target)
            idxb = np.where(fB)[0]
            rng.shuffle(idxb)
            fB[idxb[:min(drop, len(idxb))]] = False
        elif na < lo:
            drop = int(target - na)
            idxa = np.where(fA)[0]
            rng.shuffle(idxa)
            fA[idxa[:min(drop, len(idxa))]] = False
        inA[fA] = False
        inA[fB] = True
    return inA


def _host_prep(x, edge_index, W1, a_src1, a_dst1, b1, W2, a_src2, a_dst2, b2, C):
    x = np.asarray(x, np.float32)
    ei = np.asarray(edge_index)
    W1 = np.asarray(W1, np.float32)
    a_src1 = np.asarray(a_src1, np.float32)
    a_dst1 = np.asarray(a_dst1, np.float32)
    b1 = np.asarray(b1, np.float32)
    W2 = np.asarray(W2, np.float32)
    a_src2 = np.asarray(a_src2, np.float32)
    a_dst2 = np.asarray(a_dst2, np.float32)
    b2 = np.asarray(b2, np.float32)

    N, F = x.shape
    HEADS, HID = a_src1.shape
    D1 = HEADS * HID
    NCLS = W2.shape[1]
    assert N % C == 0
    NPC = N // C                      # 6250
    NT = -(-NPC // 128)               # 49
    PADN = NT * 128                   # 6272
    E = ei.shape[1]
    ET = E + N

    src = np.concatenate([ei[0], np.arange(N)]).astype(np.int64)
    dst = np.concatenate([ei[1], np.arange(N)]).astype(np.int64)
    deg = np.bincount(dst, minlength=N).astype(np.int64)

    # --- table split sizes (per-core slots; A first, then B) ---
    NT_A = (NT + 1) // 2              # 25 tiles -> 3200 slots
    NT_B = NT - NT_A                  # 24 tiles -> 3072 slots
    NA = NT_A * 128
    NB = NT_B * 128
    assert NA * C < 32768 and NB * C < 32768

    # balanced membership: per-core A count within [NPC-NB+1, NA-1]
    loA, hiA = (NPC - NB + 1) * C, (NA - 1) * C
    target = (loA + hiA) // 2
    inA = _balance_split(src, dst, deg, N, target, loA, hiA)
    dA = np.bincount(dst, weights=inA[src].astype(np.float64),
                     minlength=N).astype(np.int64)
    dB = deg - dA

    # --- tile grouping: similar (dA, dB) profiles together ---
    key = -(np.maximum(dA, dB) * 256 + np.minimum(dA, dB))
    iA = np.where(inA)[0]
    iB = np.where(~inA)[0]
    ordA = iA[np.argsort(key[iA], kind="stable")]
    ordB = iB[np.argsort(key[iB], kind="stable")]

    core_of = np.empty(N, np.int64)
    slot_of = np.empty(N, np.int64)
    jA = np.arange(len(ordA))
    core_of[ordA] = jA % C
    slot_of[ordA] = (jA // C)                      # 0..NA-1 range
    jB = np.arange(len(ordB))
    core_of[ordB] = jB % C
    slot_of[ordB] = NA + (jB // C)

    # per-tile K schedules (max over the up-to-1024 nodes of each group)
    KA = np.zeros(NT, np.int64)
    KB = np.zeros(NT, np.int64)
    for t in range(NT_A):
        g = ordA[t * 1024:(t + 1) * 1024]
        if len(g):
            KA[t] = max(dA[g].max(), 1)
            KB[t] = max(dB[g].max(), 1)
    for t in range(NT_B):
        g = ordB[t * 1024:(t + 1) * 1024]
        if len(g):
            KA[NT_A + t] = max(dA[g].max(), 1)
            KB[NT_A + t] = max(dB[g].max(), 1)
    CKA = np.concatenate([[0], np.cumsum(KA)]).astype(np.int64)
    CKB = np.concatenate([[0], np.cumsum(KB)]).astype(np.int64)
    SKA, SKB = int(CKA[-1]), int(CKB[-1])

    # table rows
    trow = np.where(inA, core_of * NA + slot_of,
                    core_of * NB + (slot_of - NA)).astype(np.int64)
    # pad rows: first unused A/B slot of core 0
    nA0 = int((core_of[ordA] == 0).sum())
    nB0 = int((core_of[ordB] == 0).sum())
    assert nA0 < NA and nB0 < NB
    padrowA = nA0            # core 0, slot nA0 (A table)
    padrowB = nB0

    # --- edge slot assignment ---
    eA = inA[src]
    eorder = np.lexsort((trow[src], (~eA).astype(np.int64), dst))
    s_src = src[eorder]
    s_dst = dst[eorder]
    s_eA = eA[eorder]
    # k index within (dst, slab): A edges sort first, so B ks = pos - dA
    startsd = np.searchsorted(s_dst, np.arange(N))
    pos_in_dst = np.arange(ET) - startsd[s_dst]
    kk = np.where(s_eA, pos_in_dst, pos_in_dst - dA[s_dst])
    assert (kk >= 0).all()

    c_d = core_of[s_dst]
    i_d = slot_of[s_dst]
    t_d = i_d // 128
    p_d = i_d % 128
    assert (np.where(s_eA, kk < KA[t_d], kk < KB[t_d])).all()

    srcgA = np.full((C, 128, SKA), padrowA, np.int16)
    srcgB = np.full((C, 128, SKB), padrowB, np.int16)
    mA = s_eA
    srcgA[c_d[mA], p_d[mA], CKA[t_d[mA]] + kk[mA]] = trow[s_src[mA]]
    mB = ~s_eA
    srcgB[c_d[mB], p_d[mB], CKB[t_d[mB]] + kk[mB]] = trow[s_src[mB]]

    # --- int16 idx tensors in dma_gather layout ---
    def mk_idx(srcg, K, CKx, SK):
        out = np.empty((C, 128, 8 * SK), np.int16)
        for t in range(NT):
            k = int(K[t])
            blk = srcg[:, :, CKx[t]:CKx[t] + k]          # [C,128,k]
            flat = blk.transpose(0, 2, 1).reshape(C, k * 128)   # i=k*128+p
            arr = flat.reshape(C, 8 * k, 16).transpose(0, 2, 1)  # [C,16,8k]
            out[:, :, 8 * CKx[t]:8 * CKx[t] + 8 * k] = np.tile(arr, (1, 8, 1))
        return out
    idxA = mk_idx(srcgA, KA, CKA, SKA)
    idxB = mk_idx(srcgB, KB, CKB, SKB)

    # --- per-core transposed x shards (pad cols zero), bf16 ---
    xs = np.zeros((C, PADN, F), np.float32)
    xs[core_of, slot_of] = x
    xsT = np.ascontiguousarray(xs.transpose(0, 2, 1)).astype(bfloat16)

    node_at = np.full((C, PADN), -1, np.int64)
    node_at[core_of, slot_of] = np.arange(N)

    # --- extended weights (f32 compute, bf16 store) ---
    Wa_s1 = np.einsum("fhc,hc->fh", W1.reshape(F, HEADS, HID), a_src1)
    Wa_d1 = np.einsum("fhc,hc->fh", W1.reshape(F, HEADS, HID), a_dst1)
    W1e = np.concatenate([W1, Wa_s1, Wa_d1], axis=1).astype(bfloat16)
    Wa_s2 = W2 @ a_src2[0]
    Wa_d2 = W2 @ a_dst2[0]
    W2e = np.concatenate([W2, Wa_s2[:, None], Wa_d2[:, None]], axis=1).astype(bfloat16)

    w2c = -W2e.astype(np.float32).sum(axis=0)
    w2cr = np.ascontiguousarray(np.broadcast_to(w2c[None, :], (128, NCLS + 2)),
                                np.float32)
    b1r = np.ascontiguousarray(np.broadcast_to(b1[None, :], (128, D1))).astype(bfloat16)
    b2r = np.ascontiguousarray(np.broadcast_to(b2[None, :], (128, NCLS)), np.float32)
    ident = np.eye(128, dtype=bfloat16)

    cfg = dict(F=F, HEADS=HEADS, HID=HID, D1=D1, NCLS=NCLS, NT=NT, NT_A=NT_A,
               PADN=PADN, NA=NA, NB=NB, C=C,
               KA=tuple(int(k) for k in KA), KB=tuple(int(k) for k in KB),
               padrowA=padrowA, padrowB=padrowB)
    in_maps = []
    for c in range(C):
        in_maps.append({
            "xsT": xsT[c],
            "w1e": W1e,
            "w2e": W2e,
            "b1r": b1r,
            "w2cr": w2cr,
            "b2r": b2r,
            "ident": ident,
            "idxA": idxA[c],
            "idxB": idxB[c],
        })
    return cfg, in_maps, node_at, (N, NCLS)


def _build_program(F, HEADS, HID, D1, NCLS, NT, NT_A, PADN, NA, NB, C, KA, KB,
                   padrowA, padrowB):
    import concourse.bacc as bacc
    import concourse.bass as bass
    import concourse.mybir as mybir
    import concourse.tile as tile
    from concourse.library_config import mlp

    f32 = mybir.dt.float32
    bf16 = mybir.dt.bfloat16
    i16 = mybir.dt.int16
    AF = mybir.ActivationFunctionType
    ALU = mybir.AluOpType
    AX = mybir.AxisListType

    TW1 = D1 + 2 * HEADS          # extended matmul width (264)
    G1W = 384                     # L1 table row (768B): [h 256 | al_src 4 | pad]
    S1W = D1 + HEADS              # scatter width (260)
    TW2 = NCLS + 2
    G2W = 128                     # L2 table row (256B): [z2 64 | al_src2 | pad]
    FK = F // 128
    DK = D1 // 128
    KM = max(KA[t] + KB[t] for t in range(NT))
    CKA = [0]
    for k in KA:
        CKA.append(CKA[-1] + k)
    CKB = [0]
    for k in KB:
        CKB.append(CKB[-1] + k)
    SKA, SKB = CKA[-1], CKB[-1]

    nc = bacc.Bacc("TRN2", target_bir_lowering=False, debug=False, num_devices=C,
                   num_swdge_queues=4)

    xsT = nc.dram_tensor("xsT", [F, PADN], bf16, kind="ExternalInput")
    w1e = nc.dram_tensor("w1e", [F, TW1], bf16, kind="ExternalInput")
    w2e = nc.dram_tensor("w2e", [D1, TW2], bf16, kind="ExternalInput")
    b1r = nc.dram_tensor("b1r", [128, D1], bf16, kind="ExternalInput")
    w2cr = nc.dram_tensor("w2cr", [128, TW2], f32, kind="ExternalInput")
    b2r = nc.dram_tensor("b2r", [128, NCLS], f32, kind="ExternalInput")
    idn = nc.dram_tensor("ident", [128, 128], bf16, kind="ExternalInput")
    idxA = nc.dram_tensor("idxA", [128, 8 * SKA], i16, kind="ExternalInput")
    idxB = nc.dram_tensor("idxB", [128, 8 * SKB], i16, kind="ExternalInput")
    outp = nc.dram_tensor("outp", [PADN, NCLS], f32, kind="ExternalOutput")

    locA1 = nc.dram_tensor("locA1", [NA, G1W], bf16)
    locB1 = nc.dram_tensor("locB1", [NB, G1W], bf16)
    tabA1 = nc.dram_tensor("tabA1", [C * NA, G1W], bf16, addr_space="Shared")
    tabB1 = nc.dram_tensor("tabB1", [C * NB, G1W], bf16, addr_space="Shared")
    locA2 = nc.dram_tensor("locA2", [NA, G2W], bf16)
    locB2 = nc.dram_tensor("locB2", [NB, G2W], bf16)
    tabA2 = nc.dram_tensor("tabA2", [C * NA, G2W], bf16, addr_space="Shared")
    tabB2 = nc.dram_tensor("tabB2", [C * NB, G2W], bf16, addr_space="Shared")

    rg = [list(range(C))]

    with tile.TileContext(nc) as tc:
        with (
            tc.tile_pool(name="const", bufs=1) as const,
            tc.tile_pool(name="alt", bufs=1) as altp,
            tc.tile_pool(name="shp", bufs=1) as shp,
            tc.tile_pool(name="g1p", bufs=3) as g1p,
            tc.tile_pool(name="g2p", bufs=3) as g2p,
            tc.tile_pool(name="g2w", bufs=2) as g2wp,
            tc.tile_pool(name="wk", bufs=2) as wk,
            tc.tile_pool(name="ep", bufs=2) as ep,
            tc.tile_pool(name="o1p", bufs=2) as o1p,
            tc.tile_pool(name="psa", bufs=3, space="PSUM") as psa,
            tc.tile_pool(name="psc", bufs=2, space="PSUM") as psc,
            tc.tile_pool(name="pst", bufs=2, space="PSUM") as pst,
            tc.tile_pool(name="psd", bufs=1, space="PSUM") as psd,
        ):
            nc.gpsimd.load_library(mlp)
            # ---- constants ----
            w1t = []
            for kk in range(FK):
                t_ = const.tile([128, TW1], bf16, tag=f"w1_{kk}")
                nc.sync.dma_start(out=t_[:], in_=w1e[kk * 128:(kk + 1) * 128, :])
                w1t.append(t_)
            w2t = []
            for kk in range(DK):
                t_ = const.tile([128, TW2], bf16, tag=f"w2_{kk}")
                nc.sync.dma_start(out=t_[:], in_=w2e[kk * 128:(kk + 1) * 128, :])
                w2t.append(t_)
            b1s = const.tile([128, D1], bf16, tag="b1")
            nc.sync.dma_start(out=b1s[:], in_=b1r[:, :])
            w2cs = const.tile([128, TW2], f32, tag="w2c")
            nc.sync.dma_start(out=w2cs[:], in_=w2cr[:, :])
            b2s = const.tile([128, NCLS], f32, tag="b2")
            nc.sync.dma_start(out=b2s[:], in_=b2r[:, :])
            ids = const.tile([128, 128], bf16, tag="ident")
            nc.sync.dma_start(out=ids[:], in_=idn[:, :])
            ixa = const.tile([128, 8 * SKA], i16, tag="ixa")
            nc.sync.dma_start(out=ixa[:], in_=idxA[:, :])
            ixb = const.tile([128, 8 * SKB], i16, tag="ixb")
            nc.sync.dma_start(out=ixb[:], in_=idxB[:, :])
            ssum = const.tile([128, NT], f32, tag="ssum")
            lgs = const.tile([128, NT], f32, tag="lgs")
            negp = const.tile([1, HEADS], bf16, tag="negp")
            nc.vector.memset(negp[:], -1e9)

            alt1 = []
            alt2 = []
            qctr = [0]

            def next_q():
                q = qctr[0] % 4
                qctr[0] += 1
                return q

            def loc_slice(tensA, tensB, t):
                if t < NT_A:
                    return tensA[t * 128:(t + 1) * 128, :]
                tb = t - NT_A
                return tensB[tb * 128:(tb + 1) * 128, :]

            # ---- phase A: local h = x @ [W1 | Wa_src | Wa_dst] ----
            # A-tiles first so AG-A overlaps the B-tile matmuls
            XG = 8
            with nc.named_scope("l1_local_mm"):
                xsup = [None] * FK
                for t in range(NT):
                    if t == NT_A:
                        nc.gpsimd.collective_compute(
                            "AllGather", mybir.AluOpType.bypass,
                            replica_groups=rg, ins=[locA1[:]], outs=[tabA1[:]])
                    if t % XG == 0:
                        w = min(XG * 128, PADN - t * 128)
                        for kk in range(FK):
                            xs_ = wk.tile([128, XG * 128], bf16, tag=f"xs{kk}")
                            nc.sync.dma_start(
                                out=xs_[:, 0:w],
                                in_=xsT[kk * 128:(kk + 1) * 128,
                                        t * 128:t * 128 + w])
                            xsup[kk] = xs_
                    xo = (t % XG) * 128
                    ps_a = psa.tile([128, TW1], f32, tag="mm")
                    for kk in range(FK):
                        nc.tensor.matmul(
                            ps_a[:], lhsT=xsup[kk][:, xo:xo + 128],
                            rhs=w1t[kk][:], start=(kk == 0), stop=(kk == FK - 1))
                    ha = wk.tile([128, TW1], bf16, tag="ha")
                    nc.scalar.copy(ha[:], ps_a[:])
                    at = altp.tile([128, HEADS], f32, tag=f"alt1_{t}")
                    nc.vector.tensor_copy(at[:], ps_a[:, D1 + HEADS:D1 + 2 * HEADS])
                    alt1.append(at)
                    nc.scalar.dma_start(out=loc_slice(locA1, locB1, t)[:, 0:S1W],
                                        in_=ha[:, 0:S1W])

            # ---- phase B: allgather layer-1 B table + pad pokes ----
            with nc.named_scope("l1_allgather"):
                nc.gpsimd.collective_compute(
                    "AllGather", mybir.AluOpType.bypass, replica_groups=rg,
                    ins=[locB1[:]], outs=[tabB1[:]],
                )
                nc.sync.dma_start(out=tabA1[padrowA:padrowA + 1, D1:D1 + HEADS],
                                  in_=negp[:])
                nc.sync.dma_start(out=tabB1[padrowB:padrowB + 1, D1:D1 + HEADS],
                                  in_=negp[:])

            # ---- phase C: layer-1 edge pass (+ fused layer-2 local mm) ----
            with nc.named_scope("l1_edges"):
                for t in range(NT):
                    Ka, Kb = KA[t], KB[t]
                    K = Ka + Kb
                    g1 = g1p.tile([128, KM * G1W], bf16, tag="g1")
                    # chunk gathers: HW limit ~1024 idxs (8 k-blocks) per call
                    for k0 in range(0, Ka, 8):
                        kc = min(8, Ka - k0)
                        gva = g1[:, k0 * G1W:(k0 + kc) * G1W].rearrange(
                            "p (k c) -> p k c", c=G1W)
                        nc.gpsimd.dma_gather(
                            gva, tabA1[:, :],
                            ixa[:, 8 * (CKA[t] + k0):8 * (CKA[t] + k0 + kc)],
                            128 * kc, 128 * kc, G1W, queue_num=next_q())
                    for k0 in range(0, Kb, 8):
                        kc = min(8, Kb - k0)
                        gvb = g1[:, (Ka + k0) * G1W:(Ka + k0 + kc) * G1W].rearrange(
                            "p (k c) -> p k c", c=G1W)
                        nc.gpsimd.dma_gather(
                            gvb, tabB1[:, :],
                            ixb[:, 8 * (CKB[t] + k0):8 * (CKB[t] + k0 + kc)],
                            128 * kc, 128 * kc, G1W, queue_num=next_q())
                    g1v = g1[:, 0:K * G1W].rearrange("p (k c) -> p k c", c=G1W)

                    # e = al_src[src] + al_dst[dst] ; ex = exp(lrelu(e))
                    e1 = ep.tile([128, KM * HEADS], f32, tag="e1")
                    altb = alt1[t][:].unsqueeze(1).to_broadcast([128, K, HEADS])
                    nc.vector.tensor_tensor(
                        e1[:, 0:K * HEADS].rearrange("p (k h) -> p k h", h=HEADS),
                        g1v[:, :, D1:D1 + HEADS], altb, op=ALU.add)
                    lr = ep.tile([128, KM * HEADS], f32, tag="lr")
                    nc.vector.tensor_scalar_mul(lr[:, 0:K * HEADS],
                                                e1[:, 0:K * HEADS], 0.2)
                    nc.vector.tensor_max(lr[:, 0:K * HEADS], lr[:, 0:K * HEADS],
                                         e1[:, 0:K * HEADS])
                    exw = ep.tile([128, KM * HEADS], bf16, tag="exw")
                    nc.scalar.activation(exw[:, 0:K * HEADS], lr[:, 0:K * HEADS],
                                         AF.Exp)
                    exv = exw[:, 0:K * HEADS].rearrange("p (k h) -> p k h", h=HEADS)

                    # weight gathered rows in place; ex into the al_src cols
                    g1f = g1v[:, :, 0:D1].rearrange("p k (h c) -> p k h c", c=HID)
                    exb = exv.unsqueeze(3).to_broadcast([128, K, HEADS, HID])
                    nc.vector.tensor_mul(g1f, g1f, exb)
                    nc.scalar.copy(g1v[:, :, D1:D1 + HEADS], exv)

                    # segment scatter-add: identity lhsT PSUM accumulation
                    ps_c = psc.tile([128, S1W], f32, tag="mm")
                    for k in range(K):
                        nc.tensor.matmul(
                            ps_c[:], lhsT=ids[:], rhs=g1v[:, k, 0:S1W],
                            start=(k == 0), stop=(k == K - 1),
                        )

                    sc1 = wk.tile([128, S1W], f32, tag="sc1")
                    nc.scalar.copy(sc1[:], ps_c[:])
                    den = ep.tile([128, HEADS], f32, tag="den")
                    nc.vector.tensor_scalar_add(den[:], sc1[:, D1:D1 + HEADS], 1e-16)
                    rec = ep.tile([128, HEADS], f32, tag="rec")
                    nc.vector.reciprocal(rec[:], den[:])

                    ob = o1p.tile([128, D1], bf16, tag="ob")
                    obv = ob[:].rearrange("p (h c) -> p h c", c=HID)
                    recb = rec[:].unsqueeze(2).to_broadcast([128, HEADS, HID])
                    psf = sc1[:, 0:D1].rearrange("p (h c) -> p h c", c=HID)
                    nc.vector.tensor_mul(obv, psf, recb)
                    nc.vector.tensor_add(ob[:], ob[:], b1s[:])
                    # elu(x)+1 = relu(x) + exp(-relu(-x)); the -1 is folded
                    # into the layer-2 matmul via the w2cs correction row
                    tn = wk.tile([128, D1], bf16, tag="tn")
                    nc.scalar.activation(tn[:], ob[:], AF.Relu, scale=-1.0)
                    nc.scalar.activation(tn[:], tn[:], AF.Exp, scale=-1.0)
                    o1 = o1p.tile([128, D1], bf16, tag="o1")
                    nc.scalar.activation(o1[:], ob[:], AF.Relu)
                    nc.vector.tensor_add(o1[:], o1[:], tn[:])

                    # fused layer-2 local mm: z2 = h2 @ [W2 | Wa2_src | Wa2_dst]
                    tts = []
                    for kk in range(DK):
                        ps_t = pst.tile([128, 128], bf16, tag="tr")
                        nc.tensor.transpose(
                            ps_t[:], o1[:, kk * 128:(kk + 1) * 128], ids[:])
                        tt = wk.tile([128, 128], bf16, tag=f"tt{kk}")
                        nc.scalar.copy(tt[:], ps_t[:])
                        tts.append(tt)
                    ps_d = psd.tile([128, TW2], f32, tag="mm")
                    for kk in range(DK):
                        nc.tensor.matmul(ps_d[:], lhsT=tts[kk][:], rhs=w2t[kk][:],
                                         start=(kk == 0), stop=(kk == DK - 1))
                    hd = wk.tile([128, TW2], bf16, tag="hd")
                    nc.vector.tensor_tensor(hd[:], ps_d[:], w2cs[:], op=ALU.add)
                    at2 = altp.tile([128, 1], f32, tag=f"alt2_{t}")
                    nc.scalar.copy(at2[:], hd[:, NCLS + 1:NCLS + 2])
                    alt2.append(at2)
                    nc.scalar.dma_start(out=loc_slice(locA2, locB2, t)[:, 0:NCLS + 1],
                                        in_=hd[:, 0:NCLS + 1])
                    if t == NT_A - 1:
                        nc.gpsimd.collective_compute(
                            "AllGather", mybir.AluOpType.bypass,
                            replica_groups=rg, ins=[locA2[:]], outs=[tabA2[:]])

            # ---- phase E: allgather layer-2 B table + pad pokes ----
            with nc.named_scope("l2_allgather"):
                nc.gpsimd.collective_compute(
                    "AllGather", mybir.AluOpType.bypass, replica_groups=rg,
                    ins=[locB2[:]], outs=[tabB2[:]],
                )
                nc.sync.dma_start(out=tabA2[padrowA:padrowA + 1, NCLS:NCLS + 1],
                                  in_=negp[:, 0:1])
                nc.sync.dma_start(out=tabB2[padrowB:padrowB + 1, NCLS:NCLS + 1],
                                  in_=negp[:, 0:1])

            # ---- phase F: layer-2 edge pass ----
            sh_tiles = []
            with nc.named_scope("l2_edges"):
                for t in range(NT):
                    Ka, Kb = KA[t], KB[t]
                    K = Ka + Kb
                    g2 = g2p.tile([128, KM * G2W], bf16, tag="g2")
                    for k0 in range(0, Ka, 8):
                        kc = min(8, Ka - k0)
                        gva = g2[:, k0 * G2W:(k0 + kc) * G2W].rearrange(
                            "p (k c) -> p k c", c=G2W)
                        nc.gpsimd.dma_gather(
                            gva, tabA2[:, :],
                            ixa[:, 8 * (CKA[t] + k0):8 * (CKA[t] + k0 + kc)],
                            128 * kc, 128 * kc, G2W, queue_num=next_q())
                    for k0 in range(0, Kb, 8):
                        kc = min(8, Kb - k0)
                        gvb = g2[:, (Ka + k0) * G2W:(Ka + k0 + kc) * G2W].rearrange(
                            "p (k c) -> p k c", c=G2W)
                        nc.gpsimd.dma_gather(
                            gvb, tabB2[:, :],
                            ixb[:, 8 * (CKB[t] + k0):8 * (CKB[t] + k0 + kc)],
                            128 * kc, 128 * kc, G2W, queue_num=next_q())
                    g2v = g2[:, 0:K * G2W].rearrange("p (k c) -> p k c", c=G2W)

                    e2 = ep.tile([128, KM], f32, tag="e2")
                    nc.vector.tensor_scalar(e2[:, 0:K], g2v[:, :, NCLS],
                                            alt2[t][:, 0:1], None, op0=ALU.add)
                    lr2 = ep.tile([128, KM], f32, tag="lr2")
                    nc.vector.tensor_scalar_mul(lr2[:, 0:K], e2[:, 0:K], 0.2)
                    nc.vector.tensor_max(lr2[:, 0:K], lr2[:, 0:K], e2[:, 0:K])
                    ex2 = ep.tile([128, KM], bf16, tag="ex2")
                    nc.scalar.activation(ex2[:, 0:K], lr2[:, 0:K], AF.Exp)

                    g2w = g2wp.tile([128, KM * NCLS], bf16, tag="g2w")
                    g2wv = g2w[:, 0:K * NCLS].rearrange("p (k c) -> p k c", c=NCLS)
                    ex2b = ex2[:, 0:K].unsqueeze(2).to_broadcast([128, K, NCLS])
                    nc.vector.tensor_mul(g2wv, g2v[:, :, 0:NCLS], ex2b)

                    ps_f = psc.tile([128, S1W], f32, tag="mm")
                    for k in range(K):
                        nc.tensor.matmul(
                            ps_f[:, 0:NCLS], lhsT=ids[:],
                            rhs=g2w[:, k * NCLS:(k + 1) * NCLS],
                            start=(k == 0), stop=(k == K - 1),
                        )

                    sc2 = wk.tile([128, NCLS + 1], f32, tag="sc2")
                    nc.scalar.copy(sc2[:, 0:NCLS], ps_f[:, 0:NCLS])
                    den2 = ep.tile([128, 1], f32, tag="den2")
                    nc.vector.reduce_sum(den2[:], ex2[:, 0:K], axis=AX.X)
                    nc.vector.tensor_scalar_add(den2[:], den2[:], 1e-16)
                    rec2 = ep.tile([128, 1], f32, tag="rec2")
                    nc.vector.reciprocal(rec2[:], den2[:])

                    o2 = wk.tile([128, NCLS], f32, tag="o2")
                    nc.vector.tensor_scalar(o2[:], sc2[:, 0:NCLS], rec2[:, 0:1],
                                            None, op0=ALU.mult)
                    nc.vector.tensor_add(o2[:], o2[:], b2s[:])

                    rmax = ep.tile([128, 1], f32, tag="rmax")
                    nc.vector.reduce_max(rmax[:], o2[:], axis=AX.X)
                    sh = shp.tile([128, NCLS], f32, tag=f"sh_{t}")
                    nc.vector.tensor_scalar(sh[:], o2[:], rmax[:, 0:1], None,
                                            op0=ALU.subtract)
                    exs = wk.tile([128, NCLS], f32, tag="exs")
                    nc.scalar.activation(exs[:], sh[:], AF.Exp,
                                         accum_out=ssum[:, t:t + 1])
                    sh_tiles.append(sh)

            # ---- phase G: log-softmax finalize ----
            with nc.named_scope("logsoftmax"):
                nc.scalar.activation(lgs[:], ssum[:], AF.Ln)
                for t in range(NT):
                    outf = wk.tile([128, NCLS], f32, tag="outf")
                    nc.vector.tensor_scalar(outf[:], sh_tiles[t][:], lgs[:, t:t + 1],
                                            None, op0=ALU.subtract)
                    nc.scalar.dma_start(out=outp[t * 128:(t + 1) * 128, :],
                                        in_=outf[:])

    nc.compile()
    return nc


def _get_program(cfg):
    key = tuple(sorted((k, v) for k, v in cfg.items()))
    if key not in _BUILD_CACHE:
        _BUILD_CACHE[key] = _build_program(**cfg)
    return _BUILD_CACHE[key]


def kernel(**inputs):
    C = 8
    cfg, in_maps, node_at, (N, NCLS) = _host_prep(
        inputs["x"], inputs["edge_index"], inputs["W1"], inputs["a_src1"],
        inputs["a_dst1"], inputs["b1"], inputs["W2"], inputs["a_src2"],
        inputs["a_dst2"], inputs["b2"], C,
    )
    nc = _get_program(cfg)

    from concourse.bass_utils import run_bass_kernel_spmd

    trace = bool(int(os.environ.get("GAT_PROFILE", "0")))
    if trace:
        trace = _register_trace_hook()
    res = run_bass_kernel_spmd(nc, in_maps, list(range(C)), trace=trace)
    if trace and res.exec_time_ns is not None:
        print(f"HW exec time: {res.exec_time_ns} ns", flush=True)

    out = np.empty((N, NCLS), np.float32)
    for c in range(C):
        r = res.results[c]["outp"]
        m = node_at[c] >= 0
        out[node_at[c][m]] = r[m]
    return out
